# revision 67
# baseline (speedup 1.0000x reference)
"""Trainium2 Bass kernel for nn_Attention_558345749040.

Reference (per batch b, H=8 heads of d=64, S=4096, E=512):
    Q = Q_seq @ WQ ; K = K_seq @ WK ; V = V_seq @ WV
    A = (Q * K) / 8                      (elementwise)
    softmax over each head's 64-wide feature group, positions j >= V_len[b]
    masked out (V_len == 0 degenerates to a uniform 1/64 softmax)
    O = softmax * V, rows s >= Q_len[b] zeroed

Structure exploited (all derived from the runtime Q_len / V_len values, so
the compiled schedule is input-shape-specialized but value-generic):
  * Rows s >= Q_len[b] are zero: only ceil(Q_len/128) 128-token chunks per
    batch carry live data. Live chunks are repartitioned evenly across the
    8 cores (token-balanced data parallel).
  * Only head positions j < V_len[b] matter: the matmuls select the 8*vl
    live weight columns through a strided AP over the shared weight tiles,
    and only packed columns are stored; the host scatters them back.
  * Narrow-width batches (vl <= 32) PACK MULTIPLE 128-token chunks into one
    PSUM slot ([128, p*8*vl] <= 512 cols): one softmax chain serves p
    chunks, amortizing the ~1us of per-slot fixed back-end cost that
    otherwise exceeds the PE's period on narrow slots. Only fully-live
    chunks pack (the shared exp-bias column must be uniform).
  * V_len == 0 batches reduce to O = V/64: V-matmul-only slots, split two
    at the front (cover the DMA runway / PE p-state ramp) and the rest at
    the back (drain the softmax pipeline under PE work).
  * fp16 transport + fp16 matmuls (rel err 3.6e-3 vs the 2e-2 gate).
  * The back-end is software-pipelined with explicit stage offsets so the
    in-order engine queues never head-of-line block:
      iter j:  PE    mmK(j) mmQ(j) mmV(j)
               ACT   k_sb(j) v_sb(j) exp(j-1)
               DVE   a(j) [mix(j)] max(j) sum(j-1) recip(j-1) ev(j-1)
               Pool  o(j-2) t_m(j)
    PSUM banks turn around within ~one iteration (psk/psv freed by the ACT
    copies, psq by the DVE a-mul at queue head), letting the PE run a full
    slot ahead. t_m is issued in the SAME iteration as max so exp(j-1)
    never waits on a same-iteration Pool op; ev = e*V runs beside
    sum/recip so stage 3 is a single Pool broadcast-multiply (o = ev * r),
    one less op on the serial chain. The final few chains run o on the
    then-idle DVE (~3x faster per element) to shorten the drain.
  * The PE runs zero-matmul warmup during the initial DMA runway: the PE
    p-state needs ~3us of continuous execution to reach max clock, so real
    matmuls start at full rate instead of paying the ramp.
  * The x gathers are CHUNK-MAJOR [128, chunk, kc, 128] so every
    chunk-range DMA piece is one contiguous >=1KB run per partition (the
    kc-major layout's 256B rows lost half the real DMA bandwidth on fine
    pieces). Startup pieces are ordered by exact first use across BOTH
    HWDGE rings: the sync ring (~300GB/s measured) carries first-vonly V,
    WV, the first regular slot's K/Q chunks, WQ, then remaining K/Q; the
    slower ACT ring (~170GB/s) carries WK at its head plus the non-vonly
    V data, qm and pm. Stores ride the sync ring (a store trigger on the
    ACT queue would steal ~0.6us from the co-critical ACT engine); the
    last sup stores in <=2-slot pieces so the tail drains incrementally.

Measured on the staged inputs: 65.9-67.0us across eight samples (from
the 80.5us recorded baseline) at rel err 3.6e-3, on an unthrottled
device; back-to-back reruns heat the shared part and can read up to
~15% higher (throttle_active_time in the NTFF summary shows it; a
baseline control run under the same conditions keeps the ~18% relative
speedup).
"""

import numpy as np
import ml_dtypes

B, S, EMB = 8, 4096, 512
H, D = 8, 64
NCORES = 8
KC = EMB // 128          # 4 contraction chunks
SUPCH = 8                # chunk columns per input-DMA superslot

_CACHE = {}


def _plan(Q_len, V_len):
    """Slot schedule shared by all cores + per-core chunk assignment.

    Returns (slots, assign, total_L, mix_L). slots[j] holds
    {kind: 'reg'|'mix'|'vonly', c, L1, p, L, off, moff, ccol}; assign[i][j]
    is a list of p entries (batch, tok0) or None (dummy).
    """
    entries = []  # (slotdict, percore list-of-lists)

    rem = []
    for b in range(B):
        ql = int(Q_len[b, 0])
        nch = -(-ql // 128) if ql > 0 else 0
        if nch == 0:
            continue
        vl = int(V_len[b, 0])
        if vl == 0:
            quota = -(-nch // NCORES)
            for t in range(quota):
                per = [[(b, (t * NCORES + i) * 128)]
                       if t * NCORES + i < nch else [None]
                       for i in range(NCORES)]
                entries.append(
                    ({"kind": "vonly", "c": D, "L1": 8 * D, "p": 1,
                      "L": 8 * D, "segs": [(D, 8 * D)]}, per))
        else:
            L1 = 8 * vl
            fq = nch // NCORES
            # slots whose 8 chunks are all fully live can pack together
            npack = 0
            while npack < fq and (npack + 1) * NCORES * 128 <= ql:
                npack += 1
            P = max(1, 512 // L1)
            t = 0
            while t < fq:
                p = min(P, npack - t) if t < npack else 1
                p = max(p, 1)
                per = [[(b, ((t + u) * NCORES + i) * 128) for u in range(p)]
                       for i in range(NCORES)]
                entries.append(
                    ({"kind": "reg", "c": vl, "L1": L1, "p": p,
                      "L": p * L1, "segs": [(vl, L1)] * p}, per))
                t += p
            for ch in range(NCORES * fq, nch):
                rem.append((vl, b, ch))

    # Remainder chunks: sort by width desc, deal round-robin into mixed
    # slots whose width is the max of their 8 chunks (others get a -1e4
    # additive mask on the padding columns).
    rem.sort(key=lambda x: -x[0])
    mixes = []
    for m in range(0, len(rem), NCORES):
        grp = rem[m:m + NCORES]
        c = grp[0][0]
        per = [[(g[1], g[2] * 128)] for g in grp]
        per += [[None]] * (NCORES - len(per))
        mixes.append(
            ({"kind": "mix", "c": c, "L1": 8 * c, "p": 1, "L": 8 * c,
              "segs": [(c, 8 * c)]}, per))
    # (Merging two narrow mix slots into one two-segment slot was tried and
    # measured 8-12us SLOWER in any schedule position: the doubled per-op
    # back-end of a merged slot exceeds every PE period.)
    entries.extend(mixes)

    # vonly at both edges; regular slots wide->narrow in between.
    von = [e for e in entries if e[0]["kind"] == "vonly"]
    reg = sorted((e for e in entries if e[0]["kind"] != "vonly"),
                 key=lambda e: -e[0]["L"])
    nvf = min(2, len(von)) if reg else len(von)
    entries = von[:nvf] + reg + von[nvf:]

    slots = [e[0] for e in entries]
    assign = [[e[1][i] for e in entries] for i in range(NCORES)]
    off = 0
    moff = 0
    ccol = 0
    for s in slots:
        s["off"] = off
        off += s["L"]
        s["ccol"] = ccol
        ccol += s["p"]
        if s["kind"] == "mix":
            s["moff"] = moff
            moff += s["L"]
    return slots, assign, off, moff


def _build(slots, total_L, mix_L):
    import concourse.bacc as bacc
    import concourse.mybir as mybir
    from concourse.tile import TileContext

    f32 = mybir.dt.float32
    f16 = mybir.dt.float16
    bf16 = mybir.dt.bfloat16
    AX = mybir.AxisListType
    OP = mybir.AluOpType
    ACTF = mybir.ActivationFunctionType

    nslot = len(slots)
    nvf = 0
    while nvf < nslot and slots[nvf]["kind"] == "vonly":
        nvf += 1
    nvb = 0
    while nvb < nslot - nvf and slots[nslot - 1 - nvb]["kind"] == "vonly":
        nvb += 1
    ncc = sum(s["p"] for s in slots)
    pmax = max(s["p"] for s in slots)

    # contiguous same-width segment runs per slot: (c, nseg, col0, g0)
    def seg_runs(s):
        runs = []
        col = 0
        g = 0
        for c, L1 in s["segs"]:
            if runs and runs[-1][0] == c:
                c0, n, col0, g0 = runs[-1]
                runs[-1] = (c0, n + 1, col0, g0)
            else:
                runs.append((c, 1, col, g))
            col += L1
            g += 8
        return runs

    # group slots into sups of <= SUPCH chunk columns
    sups = []  # (slot_lo, slot_hi, ccol0, nchunks)
    lo = 0
    while lo < nslot:
        hi = lo
        cc = 0
        while hi < nslot and cc + slots[hi]["p"] <= SUPCH:
            cc += slots[hi]["p"]
            hi += 1
        sups.append((lo, hi, slots[lo]["ccol"], cc))
        lo = hi
    nsup = len(sups)
    sup_of = {}
    for sp, (lo, hi, c0, cc) in enumerate(sups):
        for j in range(lo, hi):
            sup_of[j] = sp

    nc = bacc.Bacc()

    # x gathers are chunk-major [128 part, chunk, kc, 128]: any chunk-range
    # piece is one contiguous >=1KB run per partition, so even 1-chunk
    # pieces stream at full DMA efficiency (kc-major layout gave 256B rows)
    qg = nc.declare_dram_parameter("qg", [128, ncc * KC * 128], f16,
                                   isOutput=False)
    kg = nc.declare_dram_parameter("kg", [128, ncc * KC * 128], f16,
                                   isOutput=False)
    vg = nc.declare_dram_parameter("vg", [128, ncc * KC * 128], f16,
                                   isOutput=False)
    wq = nc.declare_dram_parameter("wq", [EMB, EMB], f16, isOutput=False)
    wk = nc.declare_dram_parameter("wk", [EMB, EMB], f16, isOutput=False)
    wv = nc.declare_dram_parameter("wv", [EMB, EMB], f16, isOutput=False)
    qm = nc.declare_dram_parameter("qm", [128, nslot], f32, isOutput=False)
    pm = (nc.declare_dram_parameter("pm", [128, mix_L], bf16, isOutput=False)
          if mix_L else None)
    outp = nc.declare_dram_parameter("outp", [128, total_L], bf16, isOutput=True)

    with TileContext(nc) as tc:
        with (
            tc.tile_pool(name="consts", bufs=1) as cpool,
            tc.tile_pool(name="xin", bufs=2) as xpool,
            tc.tile_pool(name="psq3", bufs=3, space="PSUM") as qpool,
            tc.tile_pool(name="psk2", bufs=2, space="PSUM") as kpool,
            tc.tile_pool(name="psv3", bufs=3, space="PSUM") as vpool,
            tc.tile_pool(name="work", bufs=3) as wpool,
            tc.tile_pool(name="live", bufs=4) as lpool,
            tc.tile_pool(name="outs", bufs=2) as opool,
            tc.tile_pool(name="stats", bufs=4) as spool,
        ):
            # ---- PE warmup: zero matmuls over the DMA runway so the PE
            # p-state reaches max clock before real data arrives. Tapered
            # tail so a ready real matmul waits <=~150ns.
            dummy = cpool.tile([128, EMB], f16, tag="dummy")
            nc.gpsimd.memset(dummy[:], 0.0)
            psd = qpool.tile([128, EMB], f32, tag="psq", name="psd")
            for _ in range(8):
                nc.tensor.matmul(psd[:, :EMB], dummy[:, :128], dummy[:, :EMB],
                                 start=True, stop=True)
            for _ in range(4):
                nc.tensor.matmul(psd[:, :128], dummy[:, :128], dummy[:, :128],
                                 start=True, stop=True)

            w_sb = {}

            def load_w(name, src, half=None, ring=None):
                # weights loaded in two kc-halves so the first contraction
                # chunks stream ~1us before the full tile lands (subtile
                # deps gate per-chunk).
                if name in w_sb:
                    t = w_sb[name]
                else:
                    t = cpool.tile([128, KC * EMB], f16, tag=name, name=name)
                    w_sb[name] = t
                h0, h1 = (0, KC) if half is None else half
                (ring or nc.sync).dma_start(
                    out=t[:].rearrange("p (k c) -> p k c", k=KC)[:, h0:h1, :],
                    in_=src[h0 * 128:h1 * 128, :]
                    .rearrange("(k p) c -> p k c", p=128),
                )

            def w_ap(name, kc, c):
                blk = w_sb[name][:, kc * EMB:(kc + 1) * EMB]
                if c == 64:
                    return blk
                return blk.rearrange("p (h j) -> p h j", j=D)[:, :, :c]

            def sup_tiles(sp):
                cc = sups[sp][3]
                return {name: xpool.tile([128, KC * cc * 128], f16,
                                         tag=f"x{name}", name=f"x{name}")
                        for name in ("v", "k", "q")}

            def load_piece(tiles, sp, name, src, c0, c1, ring=None):
                # load chunk-cols [c0, c1) of this sup for one tensor:
                # contiguous in both DRAM and SBUF (chunk-major layout)
                base = sups[sp][2]
                W = KC * 128
                (ring or nc.sync).dma_start(
                    out=tiles[name][:, (c0 - base) * W:(c1 - base) * W],
                    in_=src[:, c0 * W:c1 * W],
                )

            def qk_range(sp):
                lo, hi, c0, cc = sups[sp]
                a, b_ = max(lo, nvf), min(hi, nslot - nvb)
                if a >= b_:
                    return None
                return (slots[a]["ccol"],
                        slots[b_ - 1]["ccol"] + slots[b_ - 1]["p"])

            def load_sup(sp):
                lo, hi, c0, cc = sups[sp]
                tiles = sup_tiles(sp)
                load_piece(tiles, sp, "v", vg, c0, c0 + cc)
                qk = qk_range(sp)
                if qk:
                    load_piece(tiles, sp, "k", kg, qk[0], qk[1])
                    load_piece(tiles, sp, "q", qg, qk[0], qk[1])
                return tiles

            def x_ap(tiles, name, kc, j, u):
                sp = sup_of[j]
                local = slots[j]["ccol"] + u - sups[sp][2]
                col = (local * KC + kc) * 128
                return tiles[name][:, col:col + 128]

            # ---- startup loads.
            #   sync ring: leading-vonly V, qm, then K/Q/V for the rest of
            #              sup 0 (one big piece each: >=1KB descriptor rows).
            #   ACT ring:  WV, WK, WQ, pm (in parallel; the ACT queue sees
            #              no compute until ~10us in).
            # Startup: the two HWDGE rings trigger in parallel, pieces
            # ordered by first use. The sync ring (~300GB/s measured) gets
            # everything the PE's critical path needs: first-vonly V, WV,
            # the first regular slots' K/Q chunks, WK/WQ in kc-halves, then
            # the remaining K/Q. The slower ACT ring (~170GB/s) carries the
            # non-vonly V data (each slot's V matmul runs last), qm, pm.
            xs0 = sup_tiles(0)
            hi0 = sups[0][1]
            cc0 = sups[0][3]
            vsplit = min(nvf, hi0)
            if vsplit:
                load_piece(xs0, 0, "v", vg, 0, vsplit)
            load_w("wv", wv, (0, 2))
            load_w("wv", wv, (2, 4))
            # WK heads the ACT ring: the slow ring still lands it by
            # ~11.3us, before the first regular slot's K matmuls, while
            # shortening the sync ring's critical-byte queue by 512KB.
            load_w("wk", wk, (0, 2), ring=nc.scalar)
            load_w("wk", wk, (2, 4), ring=nc.scalar)
            qk0 = qk_range(0)
            if qk0:
                load_piece(xs0, 0, "k", kg, qk0[0], qk0[0] + 1)
                load_piece(xs0, 0, "q", qg, qk0[0], qk0[0] + 1)
            load_w("wq", wq, (0, 2))
            load_w("wq", wq, (2, 4))
            if qk0:
                for s0 in range(qk0[0] + 1, qk0[1], 2):
                    s1 = min(s0 + 2, qk0[1])
                    load_piece(xs0, 0, "k", kg, s0, s1)
                    load_piece(xs0, 0, "q", qg, s0, s1)
            qm_sb = cpool.tile([128, nslot], f32, tag="qm")
            if cc0 > vsplit:
                load_piece(xs0, 0, "v", vg, vsplit, min(vsplit + 2, cc0),
                           ring=nc.scalar)
            nc.scalar.dma_start(out=qm_sb[:], in_=qm[:, :])
            for s0 in range(vsplit + 2, cc0, 3):
                load_piece(xs0, 0, "v", vg, s0, min(s0 + 3, cc0),
                           ring=nc.scalar)
            if pm is not None:
                pm_sb = cpool.tile([128, mix_L], bf16, tag="pm")
                nc.scalar.dma_start(out=pm_sb[:], in_=pm[:, :])

            # ---- output staging: one [128, supL] tile per sup. Store
            # pieces: whole-sup early, <=2 slots per piece for the last sup.
            supL = [sum(slots[j]["L"] for j in range(lo, hi))
                    for lo, hi, _, _ in sups]
            oloc = {}
            for sp, (lo, hi, _, _) in enumerate(sups):
                col = 0
                for j in range(lo, hi):
                    oloc[j] = col
                    col += slots[j]["L"]
            max_supL = max(supL)
            otiles = {}

            pieces = []
            piece_of = {}
            for sp, (lo, hi, _, _) in enumerate(sups):
                rng = list(range(lo, hi))
                groups = ([rng[i:i + 2] for i in range(0, len(rng), 2)]
                          if sp == nsup - 1 else [rng])
                for g in groups:
                    for j in g:
                        piece_of[j] = len(pieces)
                    pieces.append({"sp": sp, "slots": g})

            def o_slice(j):
                sp = sup_of[j]
                if sp not in otiles:
                    otiles[sp] = opool.tile([128, max_supL], bf16, tag="osup",
                                            name="osup")
                return otiles[sp][:, oloc[j]:oloc[j] + slots[j]["L"]]

            done = set()

            def flush(j):
                done.add(j)
                pc = pieces[piece_of[j]]
                if all(k in done for k in pc["slots"]):
                    j0 = pc["slots"][0]
                    g0 = slots[j0]["off"]
                    pl = sum(slots[k]["L"] for k in pc["slots"])
                    nc.sync.dma_start(
                        out=outp[:, g0:g0 + pl],
                        in_=otiles[pc["sp"]][:, oloc[j0]:oloc[j0] + pl])

            def mm(j, xs):
                s = slots[j]
                segs = s["segs"]
                psv = vpool.tile([128, EMB], f32, tag="psv")
                st = {"psv": psv}
                tensors = [(psv, "v", "wv")]
                if s["kind"] != "vonly":
                    psk = kpool.tile([128, EMB], f32, tag="psk")
                    psq = qpool.tile([128, EMB], f32, tag="psq")
                    st["psk"], st["psq"] = psk, psq
                    tensors = [(psk, "k", "wk"), (psq, "q", "wq"),
                               (psv, "v", "wv")]
                for ps, xn, wn in tensors:
                    col = 0
                    for u, (c, L1) in enumerate(segs):
                        for kc in range(KC):
                            nc.tensor.matmul(
                                ps[:, col:col + L1],
                                x_ap(xs, xn, kc, j, u), w_ap(wn, kc, c),
                                start=(kc == 0), stop=(kc == KC - 1),
                            )
                        col += L1
                return st

            def act_copies(j, st):
                s = slots[j]
                L = s["L"]
                if s["kind"] == "vonly":
                    nc.scalar.activation(
                        o_slice(j), st["psv"][:, :L], ACTF.Copy,
                        scale=qm_sb[:, j:j + 1],
                    )
                    return
                # DVE may read at most one PSUM operand: stage K via SBUF.
                # V right behind it: the PSUM banks free within the same
                # iteration and the o-multiply runs in DVE 2x mode.
                k_sb = wpool.tile([128, EMB], f32, tag="k_sb")
                nc.scalar.copy(k_sb[:, :L], st["psk"][:, :L])
                v_sb = lpool.tile([128, EMB], f16, tag="v_sb")
                nc.scalar.copy(v_sb[:, :L], st["psv"][:, :L])
                st["k_sb"], st["v_sb"] = k_sb, v_sb

            def dve_a_max(j, st):
                s = slots[j]
                L = s["L"]
                a = wpool.tile([128, EMB], f32, tag="a")
                nc.vector.tensor_mul(a[:, :L], st["psq"][:, :L],
                                     st["k_sb"][:, :L])
                if s["kind"] == "mix":
                    moff = s["moff"]
                    am = wpool.tile([128, EMB], f32, tag="am")
                    nc.vector.scalar_tensor_tensor(
                        am[:, :L], pm_sb[:, moff:moff + L], -10000.0,
                        a[:, :L], op0=OP.mult, op1=OP.add,
                    )
                    a = am
                mneg = spool.tile([128, 8 * pmax], f32, tag="mneg")
                for c, n, col0, g0 in seg_runs(s):
                    av = (a[:, col0:col0 + n * 8 * c]
                          .rearrange("p (g j) -> p g j", j=c))
                    nc.vector.tensor_reduce(mneg[:, g0:g0 + 8 * n], av,
                                            axis=AX.X, op=OP.max, negate=True)
                st["a"], st["mneg"] = a, mneg

            def pool_tm(j, st):
                s = slots[j]
                t_m = wpool.tile([128, EMB], f32, tag="t_m")
                for c, n, col0, g0 in seg_runs(s):
                    av = (st["a"][:, col0:col0 + n * 8 * c]
                          .rearrange("p (g j) -> p g j", j=c))
                    mneg_b = (st["mneg"][:, g0:g0 + 8 * n]
                              .rearrange("p (g o) -> p g o", o=1)
                              .broadcast_to((128, 8 * n, c)))
                    nc.gpsimd.tensor_add(
                        t_m[:, col0:col0 + n * 8 * c]
                        .rearrange("p (g j) -> p g j", j=c), av, mneg_b)
                st["t_m"] = t_m

            def act_exp(j, st):
                L = slots[j]["L"]
                e = lpool.tile([128, EMB], bf16, tag="e")
                # Q_len row mask rides the exp bias: dead rows get -1e4 so
                # e == 0 there (the resulting 0*inf NaNs in dead rows are
                # zeroed by the host scatter).
                nc.scalar.activation(e[:, :L], st["t_m"][:, :L], ACTF.Exp,
                                     bias=qm_sb[:, j:j + 1])
                st["e"] = e

            def dve_sum_recip_ev(j, st):
                s = slots[j]
                L = s["L"]
                g = 8 * s["p"]
                ssum = spool.tile([128, 8 * pmax], f32, tag="ssum")
                for c, n, col0, g0 in seg_runs(s):
                    egv = (st["e"][:, col0:col0 + n * 8 * c]
                           .rearrange("p (g j) -> p g j", j=c))
                    nc.vector.tensor_reduce(ssum[:, g0:g0 + 8 * n], egv,
                                            axis=AX.X, op=OP.add)
                r = spool.tile([128, 8 * pmax], f32, tag="r")
                # ~18-bit approximate reciprocal; 1/0 on fully-dead rows
                # yields garbage that is multiplied by e == 0.
                nc.vector.reciprocal_approx_fast(r[:, :g], ssum[:, :g])
                # ev = e * V computed here (independent of r) so stage 3 is
                # a single Pool op — one less op on the serial chain. For
                # the final chains (whose o runs on the DVE) ev moves to the
                # then-idle Pool so it overlaps sum/recip instead of
                # serializing ahead of o on the DVE.
                ev = wpool.tile([128, EMB], bf16, tag="ev")
                eng = (nc.gpsimd if j >= nslot - nvb - 3 else nc.vector)
                eng.tensor_mul(ev[:, :L], st["e"][:, :L],
                               st["v_sb"][:, :L])
                st["r"], st["ev"] = r, ev

            def pool_o(j, st):
                s = slots[j]
                # Pool in steady state (offloads the DVE); DVE for the last
                # few chains, where it is idle and ~3x faster per element,
                # shortening the post-matmul drain.
                eng = (nc.vector if j >= nslot - nvb - 3 else nc.gpsimd)
                for c, n, col0, g0 in seg_runs(s):
                    evv = (st["ev"][:, col0:col0 + n * 8 * c]
                           .rearrange("p (g j) -> p g j", j=c))
                    r_b = (st["r"][:, g0:g0 + 8 * n]
                           .rearrange("p (g o) -> p g o", o=1)
                           .broadcast_to((128, 8 * n, c)))
                    eng.tensor_mul(
                        o_slice(j)[:, col0:col0 + n * 8 * c]
                        .rearrange("p (g j) -> p g j", j=c), evv, r_b)

            # ---- pipelined issue loop. Q1: slots awaiting stage 2
            # (exp/sum/recip/ev); Q2: slots awaiting stage 3 (o/store).
            # t_m rides stage 1 (issued right after max) so the ACT queue's
            # exp never waits on a same-iteration Pool op.
            xs_cur = xs0
            cur_sup = 0
            Q1, Q2 = [], []
            for j in range(nslot + 2):
                if j < nslot and sup_of[j] != cur_sup:
                    cur_sup = sup_of[j]
                    xs_cur = load_sup(cur_sup)
                s3 = Q2.pop(0) if Q2 else None
                s2 = Q1.pop(0) if Q1 else None
                st0 = None
                if j < nslot:
                    st0 = mm(j, xs_cur)               # PE
                    act_copies(j, st0)                # ACT 1,2
                reg0 = st0 is not None and slots[j]["kind"] != "vonly"
                if reg0:
                    dve_a_max(j, st0)                 # DVE 1,2[,3]
                if s3 is not None:
                    pool_o(*s3)                       # Pool 1
                    flush(s3[0])
                if s2 is not None:
                    act_exp(*s2)                      # ACT 3
                    dve_sum_recip_ev(*s2)             # DVE 4,5,6
                if reg0:
                    pool_tm(j, st0)                   # Pool 2
                    Q1.append((j, st0))
                elif st0 is not None:
                    flush(j)
                if s2 is not None:
                    Q2.append(s2)

    nc.finalize()
    return nc


def _prep_inputs(Q_seq, K_seq, V_seq, Q_len, V_len, WQ, WK, WV):
    slots, assign, total_L, mix_L = _plan(Q_len, V_len)
    f16 = np.float16
    bf = ml_dtypes.bfloat16
    nslot = len(slots)
    ncc = sum(s["p"] for s in slots)

    wq_h = np.ascontiguousarray((WQ * 0.125).astype(f16))
    wk_h = np.ascontiguousarray(WK.astype(f16))
    wv_h = np.ascontiguousarray(WV.astype(f16))

    need_qk = {ent[0] for i in range(NCORES) for j, s in enumerate(slots)
               if s["kind"] != "vonly"
               for ent in assign[i][j] if ent is not None}
    need_v = {ent[0] for i in range(NCORES) for j in range(nslot)
              for ent in assign[i][j] if ent is not None}
    qT = {b: np.ascontiguousarray(Q_seq[b].T.astype(f16)) for b in need_qk}
    kT = {b: np.ascontiguousarray(K_seq[b].T.astype(f16)) for b in need_qk}
    vT = {b: np.ascontiguousarray(V_seq[b].T.astype(f16)) for b in need_v}

    in_maps = []
    for i in range(NCORES):
        # chunk-major gather layout [128 part, chunk, kc, 128]: every
        # chunk-range DMA piece is contiguous per partition (see _build)
        qg = np.zeros((128, ncc, KC, 128), f16)
        kg = np.zeros((128, ncc, KC, 128), f16)
        vg = np.zeros((128, ncc, KC, 128), f16)
        qmv = np.zeros((128, nslot), np.float32)
        pmv = np.zeros((128, mix_L), bf) if mix_L else None
        for j, s in enumerate(slots):
            colof = 0
            for u, ent in enumerate(assign[i][j]):
                c_u, L1_u = s["segs"][u]
                if ent is not None:
                    b, tok0 = ent
                    cc = s["ccol"] + u
                    ts = slice(tok0, tok0 + 128)

                    def tile_chunk(dst, srcT):
                        dst[:, cc] = (srcT[:, ts].reshape(KC, 128, 128)
                                      .transpose(1, 0, 2))

                    tile_chunk(vg, vT[b])
                    if s["kind"] != "vonly":
                        tile_chunk(qg, qT[b])
                        tile_chunk(kg, kT[b])
                    if s["kind"] == "mix":
                        vl = int(V_len[b, 0])
                        if vl < c_u:
                            dead = np.zeros((H, c_u), np.float32)
                            dead[:, vl:] = 1.0
                            m0 = s["moff"] + colof
                            pmv[:, m0:m0 + L1_u] = np.broadcast_to(
                                dead.reshape(-1), (128, L1_u))
                colof += L1_u
            ent = assign[i][j][0]
            # Row handling is only needed on single-chunk slots: dead rows
            # produce finite garbage the host scatter never reads, so the
            # -1e4 bias is belt-and-braces; merged slots (whose chunks have
            # differing live counts) simply skip it. vonly slots need the
            # per-row 1/64 scale.
            if ent is not None and s["p"] == 1:
                b, tok0 = ent
                ql = int(Q_len[b, 0])
                live = int(np.clip(ql - tok0, 0, 128))
                if s["kind"] == "vonly":
                    # multiplicative scale on the V copy (folds the 1/64)
                    qmv[:live, j] = 1.0 / 64
                else:
                    qmv[live:, j] = -1e4
        m = {
            "qg": qg.reshape(128, ncc * KC * 128),
            "kg": kg.reshape(128, ncc * KC * 128),
            "vg": vg.reshape(128, ncc * KC * 128),
            "wq": wq_h, "wk": wk_h, "wv": wv_h,
            "qm": np.ascontiguousarray(qmv),
        }
        if mix_L:
            m["pm"] = np.ascontiguousarray(pmv)
        in_maps.append(m)
    return in_maps, slots, assign, total_L


def _run(inputs, trace=False, mm_dtype_name="", tmpdir=None):
    from concourse.bass_utils import run_bass_kernel_spmd

    Q_len = np.asarray(inputs["Q_len"])
    V_len = np.asarray(inputs["V_len"])
    in_maps, slots, assign, total_L = _prep_inputs(
        np.asarray(inputs["Q_seq"]), np.asarray(inputs["K_seq"]),
        np.asarray(inputs["V_seq"]), Q_len, V_len,
        np.asarray(inputs["WQ"]), np.asarray(inputs["WK"]),
        np.asarray(inputs["WV"]))

    key = tuple((s["kind"], tuple(s["segs"])) for s in slots)
    if key not in _CACHE:
        mix_L = sum(s["L"] for s in slots if s["kind"] == "mix")
        _CACHE[key] = _build(slots, total_L, mix_L)
    nc = _CACHE[key]

    res = run_bass_kernel_spmd(nc, in_maps, core_ids=list(range(NCORES)),
                               trace=trace, tmpdir=tmpdir)

    out = np.zeros((B, S, H * D), np.float32)
    for i in range(NCORES):
        po = res.results[i]["outp"].astype(np.float32)
        for j, s in enumerate(slots):
            off = s["off"]
            colof = 0
            for u, ent in enumerate(assign[i][j]):
                c, L1 = s["segs"][u]
                if ent is not None:
                    b, tok0 = ent
                    live = int(np.clip(int(Q_len[b, 0]) - tok0, 0, 128))
                    block = po[:live, off + colof:off + colof + L1]
                    block = block.reshape(live, H, c)
                    if s["kind"] == "vonly":
                        out[b, tok0:tok0 + live] = block.reshape(live, H * D)
                    else:
                        vl = int(V_len[b, 0])
                        out[b, tok0:tok0 + live] \
                            .reshape(live, H, D)[:, :, :vl] = block[:, :, :vl]
                colof += L1
    return out, res


def kernel(Q_seq, K_seq, V_seq, Q_len, V_len, WQ, WK, WV):
    out, _ = _run(dict(Q_seq=Q_seq, K_seq=K_seq, V_seq=V_seq,
                       Q_len=Q_len, V_len=V_len, WQ=WQ, WK=WK, WV=WV))
    return out


# revision 68
# speedup vs baseline: 1.0218x; 1.0218x over previous
"""Trainium2 Bass kernel for nn_Attention_558345749040.

Reference (per batch b, H=8 heads of d=64, S=4096, E=512):
    Q = Q_seq @ WQ ; K = K_seq @ WK ; V = V_seq @ WV
    A = (Q * K) / 8                      (elementwise)
    softmax over each head's 64-wide feature group, positions j >= V_len[b]
    masked out (V_len == 0 degenerates to a uniform 1/64 softmax)
    O = softmax * V, rows s >= Q_len[b] zeroed

Structure exploited (all derived from the runtime Q_len / V_len values, so
the compiled schedule is input-shape-specialized but value-generic):
  * Rows s >= Q_len[b] are zero: only ceil(Q_len/128) 128-token chunks per
    batch carry live data. Live chunks are repartitioned evenly across the
    8 cores (token-balanced data parallel).
  * Only head positions j < V_len[b] matter: the matmuls select the 8*vl
    live weight columns through a strided AP over the shared weight tiles,
    and only packed columns are stored; the host scatters them back.
  * Narrow-width batches (vl <= 32) PACK MULTIPLE 128-token chunks into one
    PSUM slot ([128, p*8*vl] <= 512 cols): one softmax chain serves p
    chunks, amortizing the ~1us of per-slot fixed back-end cost that
    otherwise exceeds the PE's period on narrow slots. Only fully-live
    chunks pack (the shared exp-bias column must be uniform).
  * V_len == 0 batches reduce to O = V/64: V-matmul-only slots, split two
    at the front (cover the DMA runway / PE p-state ramp) and the rest at
    the back (drain the softmax pipeline under PE work).
  * fp16 transport + fp16 matmuls (rel err 3.6e-3 vs the 2e-2 gate).
  * The back-end is software-pipelined with explicit stage offsets so the
    in-order engine queues never head-of-line block:
      iter j:  PE    mmK(j) mmQ(j) mmV(j)
               ACT   k_sb(j) v_sb(j) exp(j-1)
               DVE   a(j) [mix(j)] max(j) sum(j-1) recip(j-1) ev(j-1)
               Pool  o(j-2) t_m(j)
    PSUM banks turn around within ~one iteration (psk/psv freed by the ACT
    copies, psq by the DVE a-mul at queue head), letting the PE run a full
    slot ahead. t_m is issued in the SAME iteration as max so exp(j-1)
    never waits on a same-iteration Pool op; ev = e*V runs beside
    sum/recip so stage 3 is a single Pool broadcast-multiply (o = ev * r),
    one less op on the serial chain. The final few chains run o on the
    then-idle DVE (~3x faster per element) to shorten the drain.
  * The PE runs zero-matmul warmup during the initial DMA runway: the PE
    p-state needs ~3us of continuous execution to reach max clock, so real
    matmuls start at full rate instead of paying the ramp.
  * The x gathers are CHUNK-MAJOR [128, chunk, kc, 128] so every
    chunk-range DMA piece is one contiguous >=1KB run per partition (the
    kc-major layout's 256B rows lost half the real DMA bandwidth on fine
    pieces). Startup pieces are ordered by exact first use across BOTH
    HWDGE rings: the sync ring (~300GB/s measured) carries first-vonly V,
    WV, the first regular slot's K/Q chunks, WQ, then remaining K/Q; the
    slower ACT ring (~170GB/s) carries WK at its head plus the non-vonly
    V data, qm and pm. Stores ride the sync ring (a store trigger on the
    ACT queue would steal ~0.6us from the co-critical ACT engine); the
    last sup stores in <=2-slot pieces so the tail drains incrementally.

Measured on the staged inputs: 66.3-66.9us (from 80.5us recorded
baseline) at rel err 3.6e-3, on an unthrottled device; back-to-back
reruns heat the shared part and can read up to ~15% higher
(throttle_active_time in the NTFF summary shows it; a baseline control
run under the same conditions keeps the ~17% relative speedup).
"""

import numpy as np
import ml_dtypes

B, S, EMB = 8, 4096, 512
H, D = 8, 64
NCORES = 8
KC = EMB // 128          # 4 contraction chunks
SUPCH = 8                # chunk columns per input-DMA superslot

_CACHE = {}


def _plan(Q_len, V_len):
    """Slot schedule shared by all cores + per-core chunk assignment.

    Returns (slots, assign, total_L, mix_L). slots[j] holds
    {kind: 'reg'|'mix'|'vonly', c, L1, p, L, off, moff, ccol}; assign[i][j]
    is a list of p entries (batch, tok0) or None (dummy).
    """
    entries = []  # (slotdict, percore list-of-lists)

    rem = []
    for b in range(B):
        ql = int(Q_len[b, 0])
        nch = -(-ql // 128) if ql > 0 else 0
        if nch == 0:
            continue
        vl = int(V_len[b, 0])
        if vl == 0:
            quota = -(-nch // NCORES)
            for t in range(quota):
                per = [[(b, (t * NCORES + i) * 128)]
                       if t * NCORES + i < nch else [None]
                       for i in range(NCORES)]
                entries.append(
                    ({"kind": "vonly", "c": D, "L1": 8 * D, "p": 1,
                      "L": 8 * D, "segs": [(D, 8 * D)]}, per))
        else:
            L1 = 8 * vl
            fq = nch // NCORES
            # slots whose 8 chunks are all fully live can pack together
            npack = 0
            while npack < fq and (npack + 1) * NCORES * 128 <= ql:
                npack += 1
            P = max(1, 512 // L1)
            t = 0
            while t < fq:
                p = min(P, npack - t) if t < npack else 1
                p = max(p, 1)
                per = [[(b, ((t + u) * NCORES + i) * 128) for u in range(p)]
                       for i in range(NCORES)]
                entries.append(
                    ({"kind": "reg", "c": vl, "L1": L1, "p": p,
                      "L": p * L1, "segs": [(vl, L1)] * p}, per))
                t += p
            for ch in range(NCORES * fq, nch):
                rem.append((vl, b, ch))

    # Remainder chunks: sort by width desc, deal round-robin into mixed
    # slots whose width is the max of their 8 chunks (others get a -1e4
    # additive mask on the padding columns).
    rem.sort(key=lambda x: -x[0])
    mixes = []
    for m in range(0, len(rem), NCORES):
        grp = rem[m:m + NCORES]
        c = grp[0][0]
        per = [[(g[1], g[2] * 128)] for g in grp]
        per += [[None]] * (NCORES - len(per))
        mixes.append(
            ({"kind": "mix", "c": c, "L1": 8 * c, "p": 1, "L": 8 * c,
              "segs": [(c, 8 * c)]}, per))
    # (Merging two narrow mix slots into one two-segment slot was tried and
    # measured 8-12us SLOWER in any schedule position: the doubled per-op
    # back-end of a merged slot exceeds every PE period.)
    entries.extend(mixes)

    # vonly at both edges; regular slots wide->narrow in between.
    von = [e for e in entries if e[0]["kind"] == "vonly"]
    reg = sorted((e for e in entries if e[0]["kind"] != "vonly"),
                 key=lambda e: -e[0]["L"])
    nvf = min(2, len(von)) if reg else len(von)
    entries = von[:nvf] + reg + von[nvf:]

    slots = [e[0] for e in entries]
    assign = [[e[1][i] for e in entries] for i in range(NCORES)]
    off = 0
    moff = 0
    ccol = 0
    for s in slots:
        s["off"] = off
        off += s["L"]
        s["ccol"] = ccol
        ccol += s["p"]
        if s["kind"] == "mix":
            s["moff"] = moff
            moff += s["L"]
    return slots, assign, off, moff


def _build(slots, total_L, mix_L):
    import concourse.bacc as bacc
    import concourse.mybir as mybir
    from concourse.tile import TileContext

    f32 = mybir.dt.float32
    f16 = mybir.dt.float16
    bf16 = mybir.dt.bfloat16
    AX = mybir.AxisListType
    OP = mybir.AluOpType
    ACTF = mybir.ActivationFunctionType

    nslot = len(slots)
    nvf = 0
    while nvf < nslot and slots[nvf]["kind"] == "vonly":
        nvf += 1
    nvb = 0
    while nvb < nslot - nvf and slots[nslot - 1 - nvb]["kind"] == "vonly":
        nvb += 1
    ncc = sum(s["p"] for s in slots)
    pmax = max(s["p"] for s in slots)

    # contiguous same-width segment runs per slot: (c, nseg, col0, g0)
    def seg_runs(s):
        runs = []
        col = 0
        g = 0
        for c, L1 in s["segs"]:
            if runs and runs[-1][0] == c:
                c0, n, col0, g0 = runs[-1]
                runs[-1] = (c0, n + 1, col0, g0)
            else:
                runs.append((c, 1, col, g))
            col += L1
            g += 8
        return runs

    # group slots into sups of <= SUPCH chunk columns
    sups = []  # (slot_lo, slot_hi, ccol0, nchunks)
    lo = 0
    while lo < nslot:
        hi = lo
        cc = 0
        while hi < nslot and cc + slots[hi]["p"] <= SUPCH:
            cc += slots[hi]["p"]
            hi += 1
        sups.append((lo, hi, slots[lo]["ccol"], cc))
        lo = hi
    nsup = len(sups)
    sup_of = {}
    for sp, (lo, hi, c0, cc) in enumerate(sups):
        for j in range(lo, hi):
            sup_of[j] = sp

    nc = bacc.Bacc()

    # x gathers are chunk-major [128 part, chunk, kc, 128]: any chunk-range
    # piece is one contiguous >=1KB run per partition, so even 1-chunk
    # pieces stream at full DMA efficiency (kc-major layout gave 256B rows)
    qg = nc.declare_dram_parameter("qg", [128, ncc * KC * 128], f16,
                                   isOutput=False)
    kg = nc.declare_dram_parameter("kg", [128, ncc * KC * 128], f16,
                                   isOutput=False)
    vg = nc.declare_dram_parameter("vg", [128, ncc * KC * 128], f16,
                                   isOutput=False)
    wq = nc.declare_dram_parameter("wq", [EMB, EMB], f16, isOutput=False)
    wk = nc.declare_dram_parameter("wk", [EMB, EMB], f16, isOutput=False)
    wv = nc.declare_dram_parameter("wv", [EMB, EMB], f16, isOutput=False)
    qm = nc.declare_dram_parameter("qm", [128, nslot], f32, isOutput=False)
    pm = (nc.declare_dram_parameter("pm", [128, mix_L], bf16, isOutput=False)
          if mix_L else None)
    outp = nc.declare_dram_parameter("outp", [128, total_L], bf16, isOutput=True)

    with TileContext(nc) as tc:
        with (
            tc.tile_pool(name="consts", bufs=1) as cpool,
            tc.tile_pool(name="xin", bufs=2) as xpool,
            tc.tile_pool(name="psq3", bufs=3, space="PSUM") as qpool,
            tc.tile_pool(name="psk2", bufs=2, space="PSUM") as kpool,
            tc.tile_pool(name="psv3", bufs=3, space="PSUM") as vpool,
            tc.tile_pool(name="work", bufs=3) as wpool,
            tc.tile_pool(name="live", bufs=4) as lpool,
            tc.tile_pool(name="outs", bufs=2) as opool,
            tc.tile_pool(name="stats", bufs=4) as spool,
        ):
            # ---- PE warmup: zero matmuls over the DMA runway so the PE
            # p-state reaches max clock before real data arrives. Tapered
            # tail so a ready real matmul waits <=~150ns.
            dummy = cpool.tile([128, EMB], f16, tag="dummy")
            nc.gpsimd.memset(dummy[:], 0.0)
            psd = qpool.tile([128, EMB], f32, tag="psq", name="psd")
            for _ in range(8):
                nc.tensor.matmul(psd[:, :EMB], dummy[:, :128], dummy[:, :EMB],
                                 start=True, stop=True)
            for _ in range(4):
                nc.tensor.matmul(psd[:, :128], dummy[:, :128], dummy[:, :128],
                                 start=True, stop=True)

            w_sb = {}

            def load_w(name, src, half=None, ring=None):
                # weights loaded in two kc-halves so the first contraction
                # chunks stream ~1us before the full tile lands (subtile
                # deps gate per-chunk).
                if name in w_sb:
                    t = w_sb[name]
                else:
                    t = cpool.tile([128, KC * EMB], f16, tag=name, name=name)
                    w_sb[name] = t
                h0, h1 = (0, KC) if half is None else half
                (ring or nc.sync).dma_start(
                    out=t[:].rearrange("p (k c) -> p k c", k=KC)[:, h0:h1, :],
                    in_=src[h0 * 128:h1 * 128, :]
                    .rearrange("(k p) c -> p k c", p=128),
                )

            def w_ap(name, kc, c):
                blk = w_sb[name][:, kc * EMB:(kc + 1) * EMB]
                if c == 64:
                    return blk
                return blk.rearrange("p (h j) -> p h j", j=D)[:, :, :c]

            def sup_tiles(sp):
                cc = sups[sp][3]
                return {name: xpool.tile([128, KC * cc * 128], f16,
                                         tag=f"x{name}", name=f"x{name}")
                        for name in ("v", "k", "q")}

            def load_piece(tiles, sp, name, src, c0, c1, ring=None):
                # load chunk-cols [c0, c1) of this sup for one tensor:
                # contiguous in both DRAM and SBUF (chunk-major layout)
                base = sups[sp][2]
                W = KC * 128
                (ring or nc.sync).dma_start(
                    out=tiles[name][:, (c0 - base) * W:(c1 - base) * W],
                    in_=src[:, c0 * W:c1 * W],
                )

            def qk_range(sp):
                lo, hi, c0, cc = sups[sp]
                a, b_ = max(lo, nvf), min(hi, nslot - nvb)
                if a >= b_:
                    return None
                return (slots[a]["ccol"],
                        slots[b_ - 1]["ccol"] + slots[b_ - 1]["p"])

            def load_sup(sp):
                lo, hi, c0, cc = sups[sp]
                tiles = sup_tiles(sp)
                load_piece(tiles, sp, "v", vg, c0, c0 + cc)
                qk = qk_range(sp)
                if qk:
                    load_piece(tiles, sp, "k", kg, qk[0], qk[1])
                    load_piece(tiles, sp, "q", qg, qk[0], qk[1])
                return tiles

            def x_ap(tiles, name, kc, j, u):
                sp = sup_of[j]
                local = slots[j]["ccol"] + u - sups[sp][2]
                col = (local * KC + kc) * 128
                return tiles[name][:, col:col + 128]

            # ---- startup loads.
            #   sync ring: leading-vonly V, qm, then K/Q/V for the rest of
            #              sup 0 (one big piece each: >=1KB descriptor rows).
            #   ACT ring:  WV, WK, WQ, pm (in parallel; the ACT queue sees
            #              no compute until ~10us in).
            # Startup: the two HWDGE rings trigger in parallel, pieces
            # ordered by first use. The sync ring (~300GB/s measured) gets
            # everything the PE's critical path needs: first-vonly V, WV,
            # the first regular slots' K/Q chunks, WK/WQ in kc-halves, then
            # the remaining K/Q. The slower ACT ring (~170GB/s) carries the
            # non-vonly V data (each slot's V matmul runs last), qm, pm.
            xs0 = sup_tiles(0)
            hi0 = sups[0][1]
            cc0 = sups[0][3]
            vsplit = min(nvf, hi0)
            if vsplit:
                load_piece(xs0, 0, "v", vg, 0, vsplit)
            load_w("wv", wv, (0, 2))
            load_w("wv", wv, (2, 4))
            # WK heads the ACT ring: the slow ring still lands it by
            # ~11.3us, before the first regular slot's K matmuls, while
            # shortening the sync ring's critical-byte queue by 512KB.
            load_w("wk", wk, (0, 2), ring=nc.scalar)
            load_w("wk", wk, (2, 4), ring=nc.scalar)
            qk0 = qk_range(0)
            if qk0:
                load_piece(xs0, 0, "k", kg, qk0[0], qk0[0] + 1)
                load_piece(xs0, 0, "q", qg, qk0[0], qk0[0] + 1)
            load_w("wq", wq, (0, 2))
            load_w("wq", wq, (2, 4))
            if qk0:
                for s0 in range(qk0[0] + 1, qk0[1], 2):
                    s1 = min(s0 + 2, qk0[1])
                    load_piece(xs0, 0, "k", kg, s0, s1)
                    load_piece(xs0, 0, "q", qg, s0, s1)
            qm_sb = cpool.tile([128, nslot], f32, tag="qm")
            if cc0 > vsplit:
                load_piece(xs0, 0, "v", vg, vsplit, min(vsplit + 2, cc0),
                           ring=nc.scalar)
            nc.scalar.dma_start(out=qm_sb[:], in_=qm[:, :])
            for s0 in range(vsplit + 2, cc0, 3):
                load_piece(xs0, 0, "v", vg, s0, min(s0 + 3, cc0),
                           ring=nc.scalar)
            if pm is not None:
                pm_sb = cpool.tile([128, mix_L], bf16, tag="pm")
                nc.scalar.dma_start(out=pm_sb[:], in_=pm[:, :])

            # ---- output staging: one [128, supL] tile per sup. Store
            # pieces: whole-sup early, <=2 slots per piece for the last sup.
            supL = [sum(slots[j]["L"] for j in range(lo, hi))
                    for lo, hi, _, _ in sups]
            oloc = {}
            for sp, (lo, hi, _, _) in enumerate(sups):
                col = 0
                for j in range(lo, hi):
                    oloc[j] = col
                    col += slots[j]["L"]
            max_supL = max(supL)
            otiles = {}

            pieces = []
            piece_of = {}
            for sp, (lo, hi, _, _) in enumerate(sups):
                rng = list(range(lo, hi))
                groups = ([rng[i:i + 2] for i in range(0, len(rng), 2)]
                          if sp == nsup - 1 else [rng])
                for g in groups:
                    for j in g:
                        piece_of[j] = len(pieces)
                    pieces.append({"sp": sp, "slots": g})

            def o_slice(j):
                sp = sup_of[j]
                if sp not in otiles:
                    otiles[sp] = opool.tile([128, max_supL], bf16, tag="osup",
                                            name="osup")
                return otiles[sp][:, oloc[j]:oloc[j] + slots[j]["L"]]

            done = set()

            def flush(j):
                done.add(j)
                pc = pieces[piece_of[j]]
                if all(k in done for k in pc["slots"]):
                    j0 = pc["slots"][0]
                    g0 = slots[j0]["off"]
                    pl = sum(slots[k]["L"] for k in pc["slots"])
                    nc.sync.dma_start(
                        out=outp[:, g0:g0 + pl],
                        in_=otiles[pc["sp"]][:, oloc[j0]:oloc[j0] + pl])

            def mm(j, xs):
                s = slots[j]
                segs = s["segs"]
                psv = vpool.tile([128, EMB], f32, tag="psv")
                st = {"psv": psv}
                tensors = [(psv, "v", "wv")]
                if s["kind"] != "vonly":
                    psk = kpool.tile([128, EMB], f32, tag="psk")
                    psq = qpool.tile([128, EMB], f32, tag="psq")
                    st["psk"], st["psq"] = psk, psq
                    tensors = [(psk, "k", "wk"), (psq, "q", "wq"),
                               (psv, "v", "wv")]
                for ps, xn, wn in tensors:
                    col = 0
                    for u, (c, L1) in enumerate(segs):
                        for kc in range(KC):
                            nc.tensor.matmul(
                                ps[:, col:col + L1],
                                x_ap(xs, xn, kc, j, u), w_ap(wn, kc, c),
                                start=(kc == 0), stop=(kc == KC - 1),
                            )
                        col += L1
                return st

            def act_copies(j, st):
                s = slots[j]
                L = s["L"]
                if s["kind"] == "vonly":
                    nc.scalar.activation(
                        o_slice(j), st["psv"][:, :L], ACTF.Copy,
                        scale=qm_sb[:, j:j + 1],
                    )
                    return
                # DVE may read at most one PSUM operand: stage K via SBUF.
                # V right behind it: the PSUM banks free within the same
                # iteration and the o-multiply runs in DVE 2x mode.
                k_sb = wpool.tile([128, EMB], f32, tag="k_sb")
                nc.scalar.copy(k_sb[:, :L], st["psk"][:, :L])
                v_sb = lpool.tile([128, EMB], f16, tag="v_sb")
                nc.scalar.copy(v_sb[:, :L], st["psv"][:, :L])
                st["k_sb"], st["v_sb"] = k_sb, v_sb

            def dve_a_max(j, st):
                s = slots[j]
                L = s["L"]
                a = wpool.tile([128, EMB], f32, tag="a")
                nc.vector.tensor_mul(a[:, :L], st["psq"][:, :L],
                                     st["k_sb"][:, :L])
                if s["kind"] == "mix":
                    moff = s["moff"]
                    am = wpool.tile([128, EMB], f32, tag="am")
                    nc.vector.scalar_tensor_tensor(
                        am[:, :L], pm_sb[:, moff:moff + L], -10000.0,
                        a[:, :L], op0=OP.mult, op1=OP.add,
                    )
                    a = am
                mneg = spool.tile([128, 8 * pmax], f32, tag="mneg")
                for c, n, col0, g0 in seg_runs(s):
                    av = (a[:, col0:col0 + n * 8 * c]
                          .rearrange("p (g j) -> p g j", j=c))
                    nc.vector.tensor_reduce(mneg[:, g0:g0 + 8 * n], av,
                                            axis=AX.X, op=OP.max, negate=True)
                st["a"], st["mneg"] = a, mneg

            def pool_tm(j, st):
                s = slots[j]
                t_m = wpool.tile([128, EMB], f32, tag="t_m")
                for c, n, col0, g0 in seg_runs(s):
                    av = (st["a"][:, col0:col0 + n * 8 * c]
                          .rearrange("p (g j) -> p g j", j=c))
                    mneg_b = (st["mneg"][:, g0:g0 + 8 * n]
                              .rearrange("p (g o) -> p g o", o=1)
                              .broadcast_to((128, 8 * n, c)))
                    nc.gpsimd.tensor_add(
                        t_m[:, col0:col0 + n * 8 * c]
                        .rearrange("p (g j) -> p g j", j=c), av, mneg_b)
                st["t_m"] = t_m

            def act_exp(j, st):
                L = slots[j]["L"]
                e = lpool.tile([128, EMB], bf16, tag="e")
                # Q_len row mask rides the exp bias: dead rows get -1e4 so
                # e == 0 there (the resulting 0*inf NaNs in dead rows are
                # zeroed by the host scatter).
                nc.scalar.activation(e[:, :L], st["t_m"][:, :L], ACTF.Exp,
                                     bias=qm_sb[:, j:j + 1])
                st["e"] = e

            def dve_sum_recip_ev(j, st):
                s = slots[j]
                L = s["L"]
                g = 8 * s["p"]
                ssum = spool.tile([128, 8 * pmax], f32, tag="ssum")
                for c, n, col0, g0 in seg_runs(s):
                    egv = (st["e"][:, col0:col0 + n * 8 * c]
                           .rearrange("p (g j) -> p g j", j=c))
                    nc.vector.tensor_reduce(ssum[:, g0:g0 + 8 * n], egv,
                                            axis=AX.X, op=OP.add)
                r = spool.tile([128, 8 * pmax], f32, tag="r")
                # ~18-bit approximate reciprocal; 1/0 on fully-dead rows
                # yields garbage that is multiplied by e == 0.
                nc.vector.reciprocal_approx_fast(r[:, :g], ssum[:, :g])
                # ev = e * V computed here (independent of r) so stage 3 is
                # a single Pool op — one less op on the serial chain.
                ev = wpool.tile([128, EMB], bf16, tag="ev")
                nc.vector.tensor_mul(ev[:, :L], st["e"][:, :L],
                                     st["v_sb"][:, :L])
                st["r"], st["ev"] = r, ev

            def pool_o(j, st):
                s = slots[j]
                # Pool in steady state (offloads the DVE); DVE for the last
                # few chains, where it is idle and ~3x faster per element,
                # shortening the post-matmul drain.
                eng = (nc.vector if j >= nslot - nvb - 3 else nc.gpsimd)
                for c, n, col0, g0 in seg_runs(s):
                    evv = (st["ev"][:, col0:col0 + n * 8 * c]
                           .rearrange("p (g j) -> p g j", j=c))
                    r_b = (st["r"][:, g0:g0 + 8 * n]
                           .rearrange("p (g o) -> p g o", o=1)
                           .broadcast_to((128, 8 * n, c)))
                    eng.tensor_mul(
                        o_slice(j)[:, col0:col0 + n * 8 * c]
                        .rearrange("p (g j) -> p g j", j=c), evv, r_b)

            # ---- pipelined issue loop. Q1: slots awaiting stage 2
            # (exp/sum/recip/ev); Q2: slots awaiting stage 3 (o/store).
            # t_m rides stage 1 (issued right after max) so the ACT queue's
            # exp never waits on a same-iteration Pool op.
            xs_cur = xs0
            cur_sup = 0
            Q1, Q2 = [], []
            for j in range(nslot + 2):
                if j < nslot and sup_of[j] != cur_sup:
                    cur_sup = sup_of[j]
                    xs_cur = load_sup(cur_sup)
                s3 = Q2.pop(0) if Q2 else None
                s2 = Q1.pop(0) if Q1 else None
                st0 = None
                if j < nslot:
                    st0 = mm(j, xs_cur)               # PE
                    act_copies(j, st0)                # ACT 1,2
                reg0 = st0 is not None and slots[j]["kind"] != "vonly"
                if reg0:
                    dve_a_max(j, st0)                 # DVE 1,2[,3]
                if s3 is not None:
                    pool_o(*s3)                       # Pool 1
                    flush(s3[0])
                if s2 is not None:
                    act_exp(*s2)                      # ACT 3
                    dve_sum_recip_ev(*s2)             # DVE 4,5,6
                if reg0:
                    pool_tm(j, st0)                   # Pool 2
                    Q1.append((j, st0))
                elif st0 is not None:
                    flush(j)
                if s2 is not None:
                    Q2.append(s2)

    nc.finalize()
    return nc


def _prep_inputs(Q_seq, K_seq, V_seq, Q_len, V_len, WQ, WK, WV):
    slots, assign, total_L, mix_L = _plan(Q_len, V_len)
    f16 = np.float16
    bf = ml_dtypes.bfloat16
    nslot = len(slots)
    ncc = sum(s["p"] for s in slots)

    wq_h = np.ascontiguousarray((WQ * 0.125).astype(f16))
    wk_h = np.ascontiguousarray(WK.astype(f16))
    wv_h = np.ascontiguousarray(WV.astype(f16))

    need_qk = {ent[0] for i in range(NCORES) for j, s in enumerate(slots)
               if s["kind"] != "vonly"
               for ent in assign[i][j] if ent is not None}
    need_v = {ent[0] for i in range(NCORES) for j in range(nslot)
              for ent in assign[i][j] if ent is not None}
    qT = {b: np.ascontiguousarray(Q_seq[b].T.astype(f16)) for b in need_qk}
    kT = {b: np.ascontiguousarray(K_seq[b].T.astype(f16)) for b in need_qk}
    vT = {b: np.ascontiguousarray(V_seq[b].T.astype(f16)) for b in need_v}

    in_maps = []
    for i in range(NCORES):
        # chunk-major gather layout [128 part, chunk, kc, 128]: every
        # chunk-range DMA piece is contiguous per partition (see _build)
        qg = np.zeros((128, ncc, KC, 128), f16)
        kg = np.zeros((128, ncc, KC, 128), f16)
        vg = np.zeros((128, ncc, KC, 128), f16)
        qmv = np.zeros((128, nslot), np.float32)
        pmv = np.zeros((128, mix_L), bf) if mix_L else None
        for j, s in enumerate(slots):
            colof = 0
            for u, ent in enumerate(assign[i][j]):
                c_u, L1_u = s["segs"][u]
                if ent is not None:
                    b, tok0 = ent
                    cc = s["ccol"] + u
                    ts = slice(tok0, tok0 + 128)

                    def tile_chunk(dst, srcT):
                        dst[:, cc] = (srcT[:, ts].reshape(KC, 128, 128)
                                      .transpose(1, 0, 2))

                    tile_chunk(vg, vT[b])
                    if s["kind"] != "vonly":
                        tile_chunk(qg, qT[b])
                        tile_chunk(kg, kT[b])
                    if s["kind"] == "mix":
                        vl = int(V_len[b, 0])
                        if vl < c_u:
                            dead = np.zeros((H, c_u), np.float32)
                            dead[:, vl:] = 1.0
                            m0 = s["moff"] + colof
                            pmv[:, m0:m0 + L1_u] = np.broadcast_to(
                                dead.reshape(-1), (128, L1_u))
                colof += L1_u
            ent = assign[i][j][0]
            # Row handling is only needed on single-chunk slots: dead rows
            # produce finite garbage the host scatter never reads, so the
            # -1e4 bias is belt-and-braces; merged slots (whose chunks have
            # differing live counts) simply skip it. vonly slots need the
            # per-row 1/64 scale.
            if ent is not None and s["p"] == 1:
                b, tok0 = ent
                ql = int(Q_len[b, 0])
                live = int(np.clip(ql - tok0, 0, 128))
                if s["kind"] == "vonly":
                    # multiplicative scale on the V copy (folds the 1/64)
                    qmv[:live, j] = 1.0 / 64
                else:
                    qmv[live:, j] = -1e4
        m = {
            "qg": qg.reshape(128, ncc * KC * 128),
            "kg": kg.reshape(128, ncc * KC * 128),
            "vg": vg.reshape(128, ncc * KC * 128),
            "wq": wq_h, "wk": wk_h, "wv": wv_h,
            "qm": np.ascontiguousarray(qmv),
        }
        if mix_L:
            m["pm"] = np.ascontiguousarray(pmv)
        in_maps.append(m)
    return in_maps, slots, assign, total_L


def _run(inputs, trace=False, mm_dtype_name="", tmpdir=None):
    from concourse.bass_utils import run_bass_kernel_spmd

    Q_len = np.asarray(inputs["Q_len"])
    V_len = np.asarray(inputs["V_len"])
    in_maps, slots, assign, total_L = _prep_inputs(
        np.asarray(inputs["Q_seq"]), np.asarray(inputs["K_seq"]),
        np.asarray(inputs["V_seq"]), Q_len, V_len,
        np.asarray(inputs["WQ"]), np.asarray(inputs["WK"]),
        np.asarray(inputs["WV"]))

    key = tuple((s["kind"], tuple(s["segs"])) for s in slots)
    if key not in _CACHE:
        mix_L = sum(s["L"] for s in slots if s["kind"] == "mix")
        _CACHE[key] = _build(slots, total_L, mix_L)
    nc = _CACHE[key]

    res = run_bass_kernel_spmd(nc, in_maps, core_ids=list(range(NCORES)),
                               trace=trace, tmpdir=tmpdir)

    out = np.zeros((B, S, H * D), np.float32)
    for i in range(NCORES):
        po = res.results[i]["outp"].astype(np.float32)
        for j, s in enumerate(slots):
            off = s["off"]
            colof = 0
            for u, ent in enumerate(assign[i][j]):
                c, L1 = s["segs"][u]
                if ent is not None:
                    b, tok0 = ent
                    live = int(np.clip(int(Q_len[b, 0]) - tok0, 0, 128))
                    block = po[:live, off + colof:off + colof + L1]
                    block = block.reshape(live, H, c)
                    if s["kind"] == "vonly":
                        out[b, tok0:tok0 + live] = block.reshape(live, H * D)
                    else:
                        vl = int(V_len[b, 0])
                        out[b, tok0:tok0 + live] \
                            .reshape(live, H, D)[:, :, :vl] = block[:, :, :vl]
                colof += L1
    return out, res


def kernel(Q_seq, K_seq, V_seq, Q_len, V_len, WQ, WK, WV):
    out, _ = _run(dict(Q_seq=Q_seq, K_seq=K_seq, V_seq=V_seq,
                       Q_len=Q_len, V_len=V_len, WQ=WQ, WK=WK, WV=WV))
    return out
